# revision 10
# baseline (speedup 1.0000x reference)
"""Trainium2 Bass kernel: MultiHeadAttention (GQA + RoPE + causal), 8-core SPMD.

Sharding: 8 cores = (batch B=2) x (kv-head KVH=4). Each core handles one
(b, kvh) pair: its 4 query heads (GQA group), one K head, one V head.
Per core: Q/K/V projections in transposed [d, t] layout, rotate-half RoPE
(interleaved-pair RoPE of the reference becomes rotate-half after a head-dim
permutation of the Wq/Wk columns, applied on host), transpose-free attention
in S^T[s,t] layout with unnormalized softmax (row-sums via ones-stationary
matmuls accumulated in PSUM), row-sharded Wo producing a partial [T, C]
output in bf16. Host sums the 4 partials per batch and adds bo.

Performance structure:
- xt DRAM laid out chunk-contiguous [NTC, P, NCB*TC] -> cheap DMA triggers;
  DMA priority order wk, xt chunk 0, cos/sin, wv, wq; warm-up 14 matmuls.
- Projection/Wo matmul chains are generators, interleaved at BLOCK
  granularity into the attention loop so the PE queue always holds work
  that does not depend on the (slower) ACT exp stream.
- Causality: AV and row-sum matmuls restricted to valid columns of diagonal
  blocks; exp only on valid ranges; no pt zero-memsets.
- softmax reciprocal on ACT (DVE reciprocal is ~3.3us/tile), causal tri-mask
  multiply on Pool, Wo PSUM->SBUF copies on DVE, output DMA (bf16) triggered
  from the Pool queue.

All matmuls bf16 with fp32 PSUM accumulation.
"""

import os
import sys

for _p in ("/opt/trn_rl_repo",):
    if _p not in sys.path and os.path.isdir(_p):
        sys.path.append(_p)

import numpy as np
import ml_dtypes

import concourse.bass as bass
import concourse.mybir as mybir
from concourse import bacc
import concourse.tile as tile
from concourse.bass_utils import run_bass_kernel_spmd

BF16 = ml_dtypes.bfloat16
AF = mybir.ActivationFunctionType
F32 = mybir.dt.float32
BF = mybir.dt.bfloat16

# Problem constants (hardcoded per contract)
B, T, C = 2, 2048, 2048
H, KVH, D = 16, 4, 128
G = H // KVH          # 4 query heads per core
SCALE = D ** -0.5
THETA = 10000.0
HALF = D // 2         # 64
P = 128               # partitions
NCB = C // P          # 16 contraction blocks
TC = 512              # t-chunk (moving free dim / psum bank)
NTC = T // TC         # 4
NSB = T // P          # 16 s-blocks
NCORES = 8
NWARM = 10

_cached = {}
last_run_info = {}


def _build_bass():
    nc = bacc.Bacc(None, target_bir_lowering=False)

    xt_d = nc.dram_tensor("xt", [NTC, P, NCB * TC], BF, kind="ExternalInput")
    wq_d = nc.dram_tensor("wq", [P, NCB, G * D], BF, kind="ExternalInput")
    wk_d = nc.dram_tensor("wk", [P, NCB, D], BF, kind="ExternalInput")
    wv_d = nc.dram_tensor("wv", [P, NCB, D], BF, kind="ExternalInput")
    wo_d = nc.dram_tensor("wo", [P, G, C], BF, kind="ExternalInput")
    cos_d = nc.dram_tensor("cosb", [P, T], BF, kind="ExternalInput")
    sin_d = nc.dram_tensor("sinb", [P, T], BF, kind="ExternalInput")
    bq_d = nc.dram_tensor("bq", [G, D, 1], F32, kind="ExternalInput")
    bk_d = nc.dram_tensor("bk", [1, D, 1], F32, kind="ExternalInput")
    bv_d = nc.dram_tensor("bv", [1, D, 1], F32, kind="ExternalInput")
    tri_d = nc.dram_tensor("tri", [P, P], BF, kind="ExternalInput")
    ident_d = nc.dram_tensor("ident", [P, P], BF, kind="ExternalInput")
    out_d = nc.dram_tensor("out", [T, C], BF, kind="ExternalOutput")

    with tile.TileContext(nc) as tc:
        with (
            tc.tile_pool(name="consts", bufs=1) as consts,
            tc.tile_pool(name="wpool", bufs=1) as wpool,
            tc.tile_pool(name="qkv", bufs=1) as qkv,
            tc.tile_pool(name="psum", bufs=2, space="PSUM") as psum,
            tc.tile_pool(name="work", bufs=3) as work,
            tc.tile_pool(name="ptp", bufs=6) as ptp,
            tc.tile_pool(name="xtp", bufs=2) as xtp,
            tc.tile_pool(name="osp", bufs=2) as osp,
        ):
            # ---- tiles ----
            cos_t = consts.tile([P, T], BF)
            sin_t = consts.tile([P, T], BF)
            tri_t = consts.tile([P, P], BF)
            ident_t = consts.tile([P, P], BF)
            bq_t = consts.tile([P, G], F32)
            bk_t = consts.tile([P, 1], F32)
            bv_t = consts.tile([P, 1], F32)
            onesP = consts.tile([P, P], BF)
            scratch_t = consts.tile([P, TC], BF)

            wq_t = wpool.tile([P, NCB, G * D], BF)
            wk_t = wpool.tile([P, NCB, D], BF)
            wv_t = wpool.tile([P, NCB, D], BF)
            wo_t = wpool.tile([P, G, C], BF)

            qT = [qkv.tile([P, T], BF, name=f"qT{g}", tag=f"qT{g}") for g in range(G)]
            kT = qkv.tile([P, T], BF)
            vb = qkv.tile([P, NSB, D], BF)
            ytb = [qkv.tile([P, T], BF, name=f"yt{g}", tag=f"yt{g}") for g in range(G)]

            # ---- prologue: input DMAs in priority order, warm-up in parallel
            nc.vector.memset(scratch_t[:], 0.0)
            nc.vector.memset(onesP[:], 1.0)
            # tiny consts go on the (idle) gpsimd trigger queue
            nc.gpsimd.dma_start(ident_t[:], ident_d[:, :])
            nc.gpsimd.dma_start(tri_t[:], tri_d[:, :])
            for g in range(G):
                nc.gpsimd.dma_start(bq_t[:, g : g + 1], bq_d[g])
            nc.gpsimd.dma_start(bk_t[:], bk_d[0])
            nc.gpsimd.dma_start(bv_t[:], bv_d[0])
            # bulk inputs on sync, in first-use order
            xt_c0 = xtp.tile([P, NCB, TC], BF, tag="xt", name="xt_c0")
            nc.sync.dma_start(xt_c0[:, 0:8, :], xt_d[0][:, 0 : 8 * TC])
            nc.sync.dma_start(wk_t[:], wk_d[:, :, :])
            nc.sync.dma_start(xt_c0[:, 8:16, :], xt_d[0][:, 8 * TC : 16 * TC])
            nc.sync.dma_start(wv_t[:], wv_d[:, :, :])
            nc.sync.dma_start(cos_t[:], cos_d[:, :])
            nc.sync.dma_start(wq_t[:, 0:8, :], wq_d[:, 0:8, :])
            nc.sync.dma_start(sin_t[:], sin_d[:, :])
            nc.sync.dma_start(wq_t[:, 8:16, :], wq_d[:, 8:16, :])

            for _w in range(NWARM):
                wu_ps = psum.tile([P, TC], F32, tag="st", name="wu_ps")
                nc.tensor.matmul(wu_ps[:], scratch_t[:, 0:P], scratch_t[:],
                                 start=True, stop=True)

            # ---- generator-based matmul chains (yield after each PE op) ----
            def gen_project_rope(xt_c, w_ap_fn, bias_ap, out_tile, tcc):
                ts = slice(tcc * TC, (tcc + 1) * TC)
                ps = psum.tile([P, TC], F32, tag="proj", name="ps")
                for cb in range(NCB):
                    nc.tensor.matmul(
                        ps[:], w_ap_fn(cb), xt_c[:, cb, :],
                        start=(cb == 0), stop=(cb == NCB - 1),
                    )
                    yield
                qf = work.tile([P, TC], F32, tag="qf")
                nc.scalar.activation(qf[:], ps[:], AF.Identity, bias=bias_ap)
                sw = work.tile([P, TC], F32, tag="sw")
                nc.gpsimd.dma_start(sw[0:HALF, :], qf[HALF:P, :])
                nc.gpsimd.dma_start(sw[HALF:P, :], qf[0:HALF, :])
                t1 = work.tile([P, TC], BF, tag="t1")
                t2 = work.tile([P, TC], BF, tag="t2")
                nc.vector.tensor_mul(t1[:], qf[:], cos_t[:, ts])
                nc.vector.tensor_mul(t2[:], sw[:], sin_t[:, ts])
                nc.vector.tensor_add(out_tile[:, ts], t1[:], t2[:])

            def gen_project_v(xt_c, tcc):
                ps = psum.tile([P, TC], F32, tag="proj", name="ps")
                for cb in range(NCB):
                    nc.tensor.matmul(
                        ps[:], wv_t[:, cb, :], xt_c[:, cb, :],
                        start=(cb == 0), stop=(cb == NCB - 1),
                    )
                    yield
                vf = work.tile([P, TC], BF, tag="vf")
                nc.scalar.activation(vf[:], ps[:], AF.Identity, bias=bv_t[:, 0:1])
                for j in range(TC // P):
                    tp = psum.tile([P, P], BF, tag="proj", name="tp")
                    nc.tensor.transpose(tp[:], vf[:, j * P : (j + 1) * P], ident_t[:])
                    yield
                    nc.vector.tensor_copy(vb[:, tcc * (TC // P) + j, :], tp[:])

            def gen_wo_tb(tb):
                o_sb = osp.tile([P, C], BF, tag="osb", name="o_sb")
                for cc in range(NTC):
                    o_ps = psum.tile([P, TC], F32, tag="proj", name="o_ps")
                    for g2 in range(G):
                        nc.tensor.matmul(
                            o_ps[:],
                            ytb[g2][:, tb * P : (tb + 1) * P],
                            wo_t[:, g2, cc * TC : (cc + 1) * TC],
                            start=(g2 == 0), stop=(g2 == G - 1),
                        )
                        yield
                    nc.vector.tensor_copy(o_sb[:, cc * TC : (cc + 1) * TC], o_ps[:])
                    if cc % 2 == 1:
                        nc.gpsimd.dma_start(
                            out_d[tb * P : (tb + 1) * P, (cc - 1) * TC : (cc + 1) * TC],
                            o_sb[:, (cc - 1) * TC : (cc + 1) * TC])

            # ---- feed of pending chains, interleaved into attention ----
            feed = []          # list of (generator, approx_steps_left)
            feed_steps = [0]

            def push(gen, steps):
                feed.append(gen)
                feed_steps[0] += steps

            def adv(n):
                while n > 0 and feed:
                    try:
                        next(feed[0])
                        feed_steps[0] -= 1
                        n -= 1
                    except StopIteration:
                        feed.pop(0)

            def flush():
                while feed:
                    adv(1 << 20)

            def attn_head(g, tcc, per_block):
                ts = slice(tcc * TC, (tcc + 1) * TC)
                nsb = 4 * tcc + 4
                yt_ps = psum.tile([P, TC], F32, tag="yt", name="yt_ps")
                rs_ps = psum.tile([P, TC], F32, tag="rs", name="rs_ps")
                pts = {}

                def emit_av_rs(sb):
                    pt = pts.pop(sb)
                    c0 = max(sb * P - tcc * TC, 0)
                    nc.tensor.matmul(
                        yt_ps[:, c0:TC], vb[:, sb, :], pt[:, c0:TC],
                        start=(sb == 0), stop=(sb == nsb - 1),
                    )
                    nc.tensor.matmul(
                        rs_ps[:, c0:TC], onesP[:], pt[:, c0:TC],
                        start=(sb == 0), stop=(sb == nsb - 1),
                    )

                for sb in range(nsb):
                    st = psum.tile([P, TC], F32, tag="st", name="st")
                    r = sb * P - tcc * TC
                    c0 = max(r, 0)
                    nc.tensor.matmul(
                        st[:, c0:TC], kT[:, sb * P : (sb + 1) * P],
                        qT[g][:, tcc * TC + c0 : (tcc + 1) * TC],
                        start=True, stop=True,
                    )
                    pt = ptp.tile([P, TC], BF, tag="pt")
                    if r >= 0:
                        nc.scalar.activation(pt[:, r:TC], st[:, r:TC], AF.Exp,
                                             scale=SCALE)
                        nc.vector.tensor_mul(pt[:, r : r + P], pt[:, r : r + P],
                                             tri_t[:])
                    else:
                        nc.scalar.activation(pt[:], st[:], AF.Exp, scale=SCALE)
                    pts[sb] = pt
                    adv(per_block)
                    if sb >= 2:
                        emit_av_rs(sb - 2)
                for sb in range(max(0, nsb - 2), nsb):
                    emit_av_rs(sb)

                def finisher():
                    rb = work.tile([P, TC], F32, tag="rb")
                    nc.vector.reciprocal_approx_fast(rb[:], rs_ps[:])
                    nc.vector.tensor_mul(ytb[g][:, ts], yt_ps[:], rb[:])

                return finisher

            def load_xt(tcc):
                xt_c = xtp.tile([P, NCB, TC], BF, tag="xt", name="xt_c")
                for hh in range(2):
                    nc.sync.dma_start(
                        xt_c[:, 8 * hh : 8 * hh + 8, :],
                        xt_d[tcc][:, hh * 8 * TC : (hh + 1) * 8 * TC])
                return xt_c

            def push_proj_chunk(tcc, xt_c):
                push(gen_project_rope(xt_c, lambda cb: wk_t[:, cb, :],
                                      bk_t[:, 0:1], kT, tcc), NCB)
                push(gen_project_v(xt_c, tcc), NCB + TC // P)
                for g in range(G):
                    push(gen_project_rope(
                        xt_c,
                        lambda cb, g=g: wq_t[:, cb, g * D : (g + 1) * D],
                        bq_t[:, g : g + 1], qT[g], tcc), NCB)

            # chunk-0 bootstrap: K, V, Q0 emitted straight (attn(0,0) needs
            # only qT[0]); Q1-3 become feed for the first attention heads.
            xt_next0 = load_xt(1)
            nc.sync.dma_start(wo_t[:, 0:2, :], wo_d[:, 0:2, :])
            nc.sync.dma_start(wo_t[:, 2:4, :], wo_d[:, 2:4, :])
            push(gen_project_rope(xt_c0, lambda cb: wk_t[:, cb, :],
                                  bk_t[:, 0:1], kT, 0), NCB)
            push(gen_project_v(xt_c0, 0), NCB + TC // P)
            push(gen_project_rope(xt_c0, lambda cb: wq_t[:, cb, 0:D],
                                  bq_t[:, 0:1], qT[0], 0), NCB)
            flush()
            for g in range(1, G):
                push(gen_project_rope(
                    xt_c0,
                    lambda cb, g=g: wq_t[:, cb, g * D : (g + 1) * D],
                    bq_t[:, g : g + 1], qT[g], 0), NCB)

            pending_fin = None
            for tcc in range(NTC):
                if tcc == 0:
                    push_proj_chunk(1, xt_next0)
                elif tcc + 1 < NTC:
                    xt_next = load_xt(tcc + 1)
                    push_proj_chunk(tcc + 1, xt_next)
                nsb = 4 * tcc + 4
                for g in range(G):
                    blocks_left = (G - g) * nsb
                    per_block = -(-feed_steps[0] // blocks_left)  # ceil
                    fin = attn_head(g, tcc, per_block)
                    if pending_fin is not None:
                        pending_fin()
                    pending_fin = fin
                    if tcc > 0:
                        push(gen_wo_tb(4 * (tcc - 1) + g), NTC * G)
                flush()
            pending_fin()
            for tb in range(4 * (NTC - 1), 4 * NTC):
                push(gen_wo_tb(tb), NTC * G)
            flush()
    nc.compile()
    return nc


def _host_tables():
    perm = np.concatenate([np.arange(0, D, 2), np.arange(1, D, 2)])
    inv_freq = 1.0 / (THETA ** (np.arange(0, D, 2, dtype=np.float32) / D))
    t_idx = np.arange(T, dtype=np.float32)
    ang = t_idx[:, None] * inv_freq[None, :]          # [T, 64]
    cos_half = np.cos(ang).astype(np.float32).T       # [64, T]
    sin_half = np.sin(ang).astype(np.float32).T
    cos_b = np.concatenate([cos_half, cos_half], axis=0)       # [128, T]
    sin_b = np.concatenate([-sin_half, sin_half], axis=0)      # sign baked
    si = np.arange(P)[:, None]
    tj = np.arange(P)[None, :]
    tri = (si <= tj).astype(BF16)                      # [s, t] upper-tri incl diag
    ident = np.eye(P, dtype=BF16)
    return perm, np.ascontiguousarray(cos_b), np.ascontiguousarray(sin_b), tri, ident


def kernel(x, Wq, bq, Wk, bk, Wv, bv, Wo, bo):
    global last_run_info
    if "nc" not in _cached:
        _cached["nc"] = _build_bass()
    nc = _cached["nc"]

    x = np.asarray(x, np.float32)
    Wq = np.asarray(Wq, np.float32)
    Wk = np.asarray(Wk, np.float32)
    Wv = np.asarray(Wv, np.float32)
    Wo = np.asarray(Wo, np.float32)
    bq = np.asarray(bq, np.float32)
    bk = np.asarray(bk, np.float32)
    bv = np.asarray(bv, np.float32)
    bo = np.asarray(bo, np.float32)

    perm, cos_b, sin_b, tri, ident = _host_tables()
    cos_b = cos_b.astype(BF16)
    sin_b = sin_b.astype(BF16)

    in_maps = []
    for core in range(NCORES):
        b, kvh = divmod(core, KVH)
        # [NTC, P, NCB*TC]: xt4[tcc, p, cb*TC+tc] = x[b][tcc*TC+tc, cb*P+p]
        xt4 = np.ascontiguousarray(
            x[b].T.astype(BF16).reshape(NCB, P, NTC, TC)
            .transpose(2, 1, 0, 3).reshape(NTC, P, NCB * TC))
        qcols = np.arange(kvh * G * D, (kvh + 1) * G * D)
        wq_s = Wq[:, qcols].reshape(C, G, D)[:, :, perm].reshape(C, G * D)
        wq_s = np.ascontiguousarray(
            wq_s.astype(BF16).reshape(NCB, P, G * D).transpose(1, 0, 2))
        wk_s = np.ascontiguousarray(
            Wk[:, kvh * D : (kvh + 1) * D][:, perm].astype(BF16).reshape(NCB, P, D).transpose(1, 0, 2))
        wv_s = np.ascontiguousarray(
            Wv[:, kvh * D : (kvh + 1) * D].astype(BF16).reshape(NCB, P, D).transpose(1, 0, 2))
        wo_s = np.ascontiguousarray(
            Wo[kvh * G * D : (kvh + 1) * G * D, :].astype(BF16).reshape(G, P, C).transpose(1, 0, 2))
        bq_s = bq[qcols].reshape(G, D)[:, perm].reshape(G, D, 1).astype(np.float32)
        bk_s = bk[kvh * D : (kvh + 1) * D][perm].reshape(1, D, 1).astype(np.float32)
        bv_s = bv[kvh * D : (kvh + 1) * D].reshape(1, D, 1).astype(np.float32)
        in_maps.append({
            "xt": xt4, "wq": np.ascontiguousarray(wq_s),
            "wk": np.ascontiguousarray(wk_s), "wv": np.ascontiguousarray(wv_s),
            "wo": np.ascontiguousarray(wo_s),
            "cosb": cos_b, "sinb": sin_b,
            "bq": np.ascontiguousarray(bq_s), "bk": bk_s, "bv": bv_s,
            "tri": np.ascontiguousarray(tri), "ident": np.ascontiguousarray(ident),
        })

    try:
        res = run_bass_kernel_spmd(nc, in_maps, core_ids=list(range(NCORES)))
    except ModuleNotFoundError:
        os.environ["BASS_NEVER_TRACE"] = "1"
        res = run_bass_kernel_spmd(nc, in_maps, core_ids=list(range(NCORES)))
    last_run_info = {
        "exec_time_ns": res.exec_time_ns,
        "mean_exec_time_ns": res.mean_exec_time_ns,
        "profile_json": res.profile_json,
    }

    out = np.zeros((B, T, C), np.float32)
    for core in range(NCORES):
        b = core // KVH
        out[b] += res.results[core]["out"].astype(np.float32)
    out += bo[None, None, :]
    return out


# revision 11
# speedup vs baseline: 1.0839x; 1.0839x over previous
"""Trainium2 Bass kernel: MultiHeadAttention (GQA + RoPE + causal), 8-core SPMD.

Sharding: 8 cores = (batch B=2) x (kv-head KVH=4). Each core handles one
(b, kvh) pair: its 4 query heads (GQA group), one K head, one V head.
Per core: Q/K/V projections in transposed [d, t] layout, rotate-half RoPE
(interleaved-pair RoPE of the reference becomes rotate-half after a head-dim
permutation of the Wq/Wk columns, applied on host), transpose-free attention
in S^T[s,t] layout with unnormalized softmax (row-sums via ones-stationary
matmuls accumulated in PSUM), row-sharded Wo producing a partial [T, C]
output in bf16. Host sums the 4 partials per batch and adds bo.

Performance structure:
- xt DRAM laid out chunk-contiguous [NTC, P, NCB*TC] -> cheap DMA triggers;
  DMA priority order wk, xt chunk 0, cos/sin, wv, wq; warm-up 14 matmuls.
- Projection/Wo matmul chains are generators, interleaved at BLOCK
  granularity into the attention loop so the PE queue always holds work
  that does not depend on the (slower) ACT exp stream.
- Causality: AV and row-sum matmuls restricted to valid columns of diagonal
  blocks; exp only on valid ranges; no pt zero-memsets.
- softmax reciprocal on ACT (DVE reciprocal is ~3.3us/tile), causal tri-mask
  multiply on Pool, Wo PSUM->SBUF copies on DVE, output DMA (bf16) triggered
  from the Pool queue.

All matmuls bf16 with fp32 PSUM accumulation.
"""

import os
import sys

for _p in ("/opt/trn_rl_repo",):
    if _p not in sys.path and os.path.isdir(_p):
        sys.path.append(_p)

import numpy as np
import ml_dtypes

import concourse.bass as bass
import concourse.mybir as mybir
from concourse import bacc
import concourse.tile as tile
from concourse.bass_utils import run_bass_kernel_spmd

BF16 = ml_dtypes.bfloat16
AF = mybir.ActivationFunctionType
F32 = mybir.dt.float32
BF = mybir.dt.bfloat16

# Problem constants (hardcoded per contract)
B, T, C = 2, 2048, 2048
H, KVH, D = 16, 4, 128
G = H // KVH          # 4 query heads per core
SCALE = D ** -0.5
THETA = 10000.0
HALF = D // 2         # 64
P = 128               # partitions
NCB = C // P          # 16 contraction blocks
TC = 512              # t-chunk (moving free dim / psum bank)
NTC = T // TC         # 4
NSB = T // P          # 16 s-blocks
NCORES = 8
NWARM = 10

_cached = {}
last_run_info = {}


def _build_bass():
    nc = bacc.Bacc(None, target_bir_lowering=False)

    xt_d = nc.dram_tensor("xt", [NTC, P, NCB * TC], BF, kind="ExternalInput")
    wq_d = nc.dram_tensor("wq", [P, NCB, G * D], BF, kind="ExternalInput")
    wk_d = nc.dram_tensor("wk", [P, NCB, D], BF, kind="ExternalInput")
    wv_d = nc.dram_tensor("wv", [P, NCB, D], BF, kind="ExternalInput")
    wo_d = nc.dram_tensor("wo", [P, G, C], BF, kind="ExternalInput")
    cos_d = nc.dram_tensor("cosb", [P, T], BF, kind="ExternalInput")
    sin_d = nc.dram_tensor("sinb", [P, T], BF, kind="ExternalInput")
    bq_d = nc.dram_tensor("bq", [G, D, 1], F32, kind="ExternalInput")
    bk_d = nc.dram_tensor("bk", [1, D, 1], F32, kind="ExternalInput")
    bv_d = nc.dram_tensor("bv", [1, D, 1], F32, kind="ExternalInput")
    tri_d = nc.dram_tensor("tri", [P, P], BF, kind="ExternalInput")
    ident_d = nc.dram_tensor("ident", [P, P], BF, kind="ExternalInput")
    out_d = nc.dram_tensor("out", [T, C], BF, kind="ExternalOutput")

    with tile.TileContext(nc) as tc:
        with (
            tc.tile_pool(name="consts", bufs=1) as consts,
            tc.tile_pool(name="wpool", bufs=1) as wpool,
            tc.tile_pool(name="qkv", bufs=1) as qkv,
            tc.tile_pool(name="psum", bufs=2, space="PSUM") as psum,
            tc.tile_pool(name="work", bufs=3) as work,
            tc.tile_pool(name="ptp", bufs=6) as ptp,
            tc.tile_pool(name="xtp", bufs=2) as xtp,
            tc.tile_pool(name="osp", bufs=2) as osp,
        ):
            # ---- tiles ----
            cos_t = consts.tile([P, T], BF)
            sin_t = consts.tile([P, T], BF)
            tri_t = consts.tile([P, P], BF)
            ident_t = consts.tile([P, P], BF)
            bq_t = consts.tile([P, G], F32)
            bk_t = consts.tile([P, 1], F32)
            bv_t = consts.tile([P, 1], F32)
            onesP = consts.tile([P, P], BF)
            scratch_t = consts.tile([P, TC], BF)

            wq_t = wpool.tile([P, NCB, G * D], BF)
            wk_t = wpool.tile([P, NCB, D], BF)
            wv_t = wpool.tile([P, NCB, D], BF)
            wo_t = wpool.tile([P, G, C], BF)

            qT = [qkv.tile([P, T], BF, name=f"qT{g}", tag=f"qT{g}") for g in range(G)]
            kT = qkv.tile([P, T], BF)
            vb = qkv.tile([P, NSB, D], BF)
            ytb = [qkv.tile([P, T], BF, name=f"yt{g}", tag=f"yt{g}") for g in range(G)]

            # ---- prologue: input DMAs in priority order, warm-up in parallel
            nc.vector.memset(scratch_t[:], 0.0)
            nc.vector.memset(onesP[:], 1.0)
            # tiny consts go on the (idle) gpsimd trigger queue
            nc.gpsimd.dma_start(ident_t[:], ident_d[:, :])
            nc.gpsimd.dma_start(tri_t[:], tri_d[:, :])
            for g in range(G):
                nc.gpsimd.dma_start(bq_t[:, g : g + 1], bq_d[g])
            nc.gpsimd.dma_start(bk_t[:], bk_d[0])
            nc.gpsimd.dma_start(bv_t[:], bv_d[0])
            # bulk inputs on sync, in first-use order
            xt_c0 = xtp.tile([P, NCB, TC], BF, tag="xt", name="xt_c0")
            nc.sync.dma_start(xt_c0[:, 0:8, :], xt_d[0][:, 0 : 8 * TC])
            nc.sync.dma_start(wk_t[:], wk_d[:, :, :])
            nc.sync.dma_start(xt_c0[:, 8:16, :], xt_d[0][:, 8 * TC : 16 * TC])
            nc.sync.dma_start(wv_t[:], wv_d[:, :, :])
            nc.sync.dma_start(cos_t[:], cos_d[:, :])
            nc.sync.dma_start(wq_t[:, 0:8, :], wq_d[:, 0:8, :])
            nc.sync.dma_start(sin_t[:], sin_d[:, :])
            nc.sync.dma_start(wq_t[:, 8:16, :], wq_d[:, 8:16, :])

            for _w in range(NWARM):
                wu_ps = psum.tile([P, TC], F32, tag="st", name="wu_ps")
                nc.tensor.matmul(wu_ps[:], scratch_t[:, 0:P], scratch_t[:],
                                 start=True, stop=True)

            # ---- generator-based matmul chains (yield after each PE op) ----
            def gen_project_rope(xt_c, w_ap_fn, bias_ap, out_tile, tcc):
                ts = slice(tcc * TC, (tcc + 1) * TC)
                ps = psum.tile([P, TC], F32, tag="proj", name="ps")
                for cb in range(NCB):
                    nc.tensor.matmul(
                        ps[:], w_ap_fn(cb), xt_c[:, cb, :],
                        start=(cb == 0), stop=(cb == NCB - 1),
                    )
                    yield
                qf = work.tile([P, TC], F32, tag="qf")
                nc.scalar.activation(qf[:], ps[:], AF.Identity, bias=bias_ap)
                sw = work.tile([P, TC], F32, tag="sw")
                nc.sync.dma_start(sw[0:HALF, :], qf[HALF:P, :])
                nc.sync.dma_start(sw[HALF:P, :], qf[0:HALF, :])
                t1 = work.tile([P, TC], BF, tag="t1")
                t2 = work.tile([P, TC], BF, tag="t2")
                nc.vector.tensor_mul(t1[:], qf[:], cos_t[:, ts])
                nc.vector.tensor_mul(t2[:], sw[:], sin_t[:, ts])
                nc.vector.tensor_add(out_tile[:, ts], t1[:], t2[:])

            def gen_project_v(xt_c, tcc):
                ps = psum.tile([P, TC], F32, tag="proj", name="ps")
                for cb in range(NCB):
                    nc.tensor.matmul(
                        ps[:], wv_t[:, cb, :], xt_c[:, cb, :],
                        start=(cb == 0), stop=(cb == NCB - 1),
                    )
                    yield
                vf = work.tile([P, TC], BF, tag="vf")
                nc.scalar.activation(vf[:], ps[:], AF.Identity, bias=bv_t[:, 0:1])
                for j in range(TC // P):
                    tp = psum.tile([P, P], BF, tag="proj", name="tp")
                    nc.tensor.transpose(tp[:], vf[:, j * P : (j + 1) * P], ident_t[:])
                    yield
                    nc.vector.tensor_copy(vb[:, tcc * (TC // P) + j, :], tp[:])

            def gen_wo_tb(tb):
                o_sb = osp.tile([P, C], BF, tag="osb", name="o_sb")
                for cc in range(NTC):
                    o_ps = psum.tile([P, TC], F32, tag="proj", name="o_ps")
                    for g2 in range(G):
                        nc.tensor.matmul(
                            o_ps[:],
                            ytb[g2][:, tb * P : (tb + 1) * P],
                            wo_t[:, g2, cc * TC : (cc + 1) * TC],
                            start=(g2 == 0), stop=(g2 == G - 1),
                        )
                        yield
                    nc.vector.tensor_copy(o_sb[:, cc * TC : (cc + 1) * TC], o_ps[:])
                    if cc % 2 == 1:
                        nc.gpsimd.dma_start(
                            out_d[tb * P : (tb + 1) * P, (cc - 1) * TC : (cc + 1) * TC],
                            o_sb[:, (cc - 1) * TC : (cc + 1) * TC])

            # ---- feed of pending chains, interleaved into attention ----
            feed = []          # list of (generator, approx_steps_left)
            feed_steps = [0]

            def push(gen, steps):
                feed.append(gen)
                feed_steps[0] += steps

            def adv(n):
                while n > 0 and feed:
                    try:
                        next(feed[0])
                        feed_steps[0] -= 1
                        n -= 1
                    except StopIteration:
                        feed.pop(0)

            def flush():
                while feed:
                    adv(1 << 20)

            def attn_head(g, tcc, per_block):
                ts = slice(tcc * TC, (tcc + 1) * TC)
                nsb = 4 * tcc + 4
                yt_ps = psum.tile([P, TC], F32, tag="yt", name="yt_ps")
                rs_ps = psum.tile([P, TC], F32, tag="rs", name="rs_ps")
                pts = {}

                def emit_av_rs(sb):
                    pt = pts.pop(sb)
                    c0 = max(sb * P - tcc * TC, 0)
                    nc.tensor.matmul(
                        yt_ps[:, c0:TC], vb[:, sb, :], pt[:, c0:TC],
                        start=(sb == 0), stop=(sb == nsb - 1),
                    )
                    nc.tensor.matmul(
                        rs_ps[:, c0:TC], onesP[:], pt[:, c0:TC],
                        start=(sb == 0), stop=(sb == nsb - 1),
                    )

                for sb in range(nsb):
                    st = psum.tile([P, TC], F32, tag="st", name="st")
                    r = sb * P - tcc * TC
                    c0 = max(r, 0)
                    nc.tensor.matmul(
                        st[:, c0:TC], kT[:, sb * P : (sb + 1) * P],
                        qT[g][:, tcc * TC + c0 : (tcc + 1) * TC],
                        start=True, stop=True,
                    )
                    pt = ptp.tile([P, TC], BF, tag="pt")
                    if r >= 0:
                        nc.scalar.activation(pt[:, r:TC], st[:, r:TC], AF.Exp,
                                             scale=SCALE)
                        nc.vector.tensor_mul(pt[:, r : r + P], pt[:, r : r + P],
                                             tri_t[:])
                    else:
                        nc.scalar.activation(pt[:], st[:], AF.Exp, scale=SCALE)
                    pts[sb] = pt
                    adv(per_block)
                    if sb >= 2:
                        emit_av_rs(sb - 2)
                for sb in range(max(0, nsb - 2), nsb):
                    emit_av_rs(sb)

                def finisher():
                    rb = work.tile([P, TC], F32, tag="rb")
                    nc.vector.reciprocal_approx_fast(rb[:], rs_ps[:])
                    nc.vector.tensor_mul(ytb[g][:, ts], yt_ps[:], rb[:])

                return finisher

            def load_xt(tcc):
                xt_c = xtp.tile([P, NCB, TC], BF, tag="xt", name="xt_c")
                for hh in range(2):
                    nc.sync.dma_start(
                        xt_c[:, 8 * hh : 8 * hh + 8, :],
                        xt_d[tcc][:, hh * 8 * TC : (hh + 1) * 8 * TC])
                return xt_c

            def push_proj_chunk(tcc, xt_c):
                push(gen_project_rope(xt_c, lambda cb: wk_t[:, cb, :],
                                      bk_t[:, 0:1], kT, tcc), NCB)
                push(gen_project_v(xt_c, tcc), NCB + TC // P)
                for g in range(G):
                    push(gen_project_rope(
                        xt_c,
                        lambda cb, g=g: wq_t[:, cb, g * D : (g + 1) * D],
                        bq_t[:, g : g + 1], qT[g], tcc), NCB)

            # chunk-0 bootstrap: K, V, Q0 emitted straight (attn(0,0) needs
            # only qT[0]); Q1-3 become feed for the first attention heads.
            push(gen_project_rope(xt_c0, lambda cb: wk_t[:, cb, :],
                                  bk_t[:, 0:1], kT, 0), NCB)
            push(gen_project_v(xt_c0, 0), NCB + TC // P)
            push(gen_project_rope(xt_c0, lambda cb: wq_t[:, cb, 0:D],
                                  bq_t[:, 0:1], qT[0], 0), NCB)
            flush()
            xt_next0 = load_xt(1)
            nc.sync.dma_start(wo_t[:, 0:2, :], wo_d[:, 0:2, :])
            nc.sync.dma_start(wo_t[:, 2:4, :], wo_d[:, 2:4, :])
            for g in range(1, G):
                push(gen_project_rope(
                    xt_c0,
                    lambda cb, g=g: wq_t[:, cb, g * D : (g + 1) * D],
                    bq_t[:, g : g + 1], qT[g], 0), NCB)

            pending_fin = None
            for tcc in range(NTC):
                if tcc == 0:
                    push_proj_chunk(1, xt_next0)
                elif tcc + 1 < NTC:
                    xt_next = load_xt(tcc + 1)
                    push_proj_chunk(tcc + 1, xt_next)
                nsb = 4 * tcc + 4
                for g in range(G):
                    blocks_left = (G - g) * nsb
                    per_block = -(-feed_steps[0] // blocks_left)  # ceil
                    fin = attn_head(g, tcc, per_block)
                    if pending_fin is not None:
                        pending_fin()
                    pending_fin = fin
                    if tcc > 0:
                        push(gen_wo_tb(4 * (tcc - 1) + g), NTC * G)
                flush()
            pending_fin()
            for tb in range(4 * (NTC - 1), 4 * NTC):
                push(gen_wo_tb(tb), NTC * G)
            flush()
    nc.compile()
    return nc


def _host_tables():
    perm = np.concatenate([np.arange(0, D, 2), np.arange(1, D, 2)])
    inv_freq = 1.0 / (THETA ** (np.arange(0, D, 2, dtype=np.float32) / D))
    t_idx = np.arange(T, dtype=np.float32)
    ang = t_idx[:, None] * inv_freq[None, :]          # [T, 64]
    cos_half = np.cos(ang).astype(np.float32).T       # [64, T]
    sin_half = np.sin(ang).astype(np.float32).T
    cos_b = np.concatenate([cos_half, cos_half], axis=0)       # [128, T]
    sin_b = np.concatenate([-sin_half, sin_half], axis=0)      # sign baked
    si = np.arange(P)[:, None]
    tj = np.arange(P)[None, :]
    tri = (si <= tj).astype(BF16)                      # [s, t] upper-tri incl diag
    ident = np.eye(P, dtype=BF16)
    return perm, np.ascontiguousarray(cos_b), np.ascontiguousarray(sin_b), tri, ident


def kernel(x, Wq, bq, Wk, bk, Wv, bv, Wo, bo):
    global last_run_info
    if "nc" not in _cached:
        _cached["nc"] = _build_bass()
    nc = _cached["nc"]

    x = np.asarray(x, np.float32)
    Wq = np.asarray(Wq, np.float32)
    Wk = np.asarray(Wk, np.float32)
    Wv = np.asarray(Wv, np.float32)
    Wo = np.asarray(Wo, np.float32)
    bq = np.asarray(bq, np.float32)
    bk = np.asarray(bk, np.float32)
    bv = np.asarray(bv, np.float32)
    bo = np.asarray(bo, np.float32)

    perm, cos_b, sin_b, tri, ident = _host_tables()
    cos_b = cos_b.astype(BF16)
    sin_b = sin_b.astype(BF16)

    in_maps = []
    for core in range(NCORES):
        b, kvh = divmod(core, KVH)
        # [NTC, P, NCB*TC]: xt4[tcc, p, cb*TC+tc] = x[b][tcc*TC+tc, cb*P+p]
        xt4 = np.ascontiguousarray(
            x[b].T.astype(BF16).reshape(NCB, P, NTC, TC)
            .transpose(2, 1, 0, 3).reshape(NTC, P, NCB * TC))
        qcols = np.arange(kvh * G * D, (kvh + 1) * G * D)
        wq_s = Wq[:, qcols].reshape(C, G, D)[:, :, perm].reshape(C, G * D)
        wq_s = np.ascontiguousarray(
            wq_s.astype(BF16).reshape(NCB, P, G * D).transpose(1, 0, 2))
        wk_s = np.ascontiguousarray(
            Wk[:, kvh * D : (kvh + 1) * D][:, perm].astype(BF16).reshape(NCB, P, D).transpose(1, 0, 2))
        wv_s = np.ascontiguousarray(
            Wv[:, kvh * D : (kvh + 1) * D].astype(BF16).reshape(NCB, P, D).transpose(1, 0, 2))
        wo_s = np.ascontiguousarray(
            Wo[kvh * G * D : (kvh + 1) * G * D, :].astype(BF16).reshape(G, P, C).transpose(1, 0, 2))
        bq_s = bq[qcols].reshape(G, D)[:, perm].reshape(G, D, 1).astype(np.float32)
        bk_s = bk[kvh * D : (kvh + 1) * D][perm].reshape(1, D, 1).astype(np.float32)
        bv_s = bv[kvh * D : (kvh + 1) * D].reshape(1, D, 1).astype(np.float32)
        in_maps.append({
            "xt": xt4, "wq": np.ascontiguousarray(wq_s),
            "wk": np.ascontiguousarray(wk_s), "wv": np.ascontiguousarray(wv_s),
            "wo": np.ascontiguousarray(wo_s),
            "cosb": cos_b, "sinb": sin_b,
            "bq": np.ascontiguousarray(bq_s), "bk": bk_s, "bv": bv_s,
            "tri": np.ascontiguousarray(tri), "ident": np.ascontiguousarray(ident),
        })

    try:
        res = run_bass_kernel_spmd(nc, in_maps, core_ids=list(range(NCORES)))
    except ModuleNotFoundError:
        os.environ["BASS_NEVER_TRACE"] = "1"
        res = run_bass_kernel_spmd(nc, in_maps, core_ids=list(range(NCORES)))
    last_run_info = {
        "exec_time_ns": res.exec_time_ns,
        "mean_exec_time_ns": res.mean_exec_time_ns,
        "profile_json": res.profile_json,
    }

    out = np.zeros((B, T, C), np.float32)
    for core in range(NCORES):
        b = core // KVH
        out[b] += res.results[core]["out"].astype(np.float32)
    out += bo[None, None, :]
    return out
